# revision 1
# baseline (speedup 1.0000x reference)
"""BitMLP (BitNet-style MLP) Trainium2 kernel, 8-way data-parallel over tokens.

reference semantics:
  h   = act_quant(rms_norm(x, gamma)) @ w1q.T   (w1q = per-tensor ternary quant)
  out = act_quant(gelu_exact(h)) @ w2q.T

Key facts exploited:
  * act_quant produces n/scale with n an integer in [-127, 127]  -> n is exact in bf16
  * weight quant produces t*inv_w with t ternary in {-1, 0, 1}   -> t is exact in bf16
  * therefore both matmuls are exact integer accumulations computed in bf16 at
    full TensorE rate; per-token/per-tensor scales are applied afterwards.

Sharding (8 cores on one chip):
  * tokens (4*2048 = 8192) split 1024/core; each core computes its tokens' output
  * weight quantization is cooperative: core c quantizes 1/8 of w1 and w2,
    the per-tensor mean(|w|) is combined with a scalar AllReduce, and the
    ternary bf16 weights are AllGathered.

v2 schedule (from trace analysis of v1):
  * the X phase (rms-norm stats + activation quant) runs CONCURRENTLY with the
    weight-stats pass and the AllReduce, so nxT is ready ~when AG chunk 0 lands
    (v1 serialized X after all weight AllGathers: 480us of PE idle).
  * w1's AllGather is chunked unevenly [1,2,2,3] hid-blocks/core: chunk 0 is
    tiny so MM1 starts ASAP; later chunks hide under MM1.
  * h lives in SBUF as bf16 (v1 bounced 64MB of f32 through DRAM); Q2
    requantizes it in-place, halves pipelined so Q2(half1) hides under
    MM2(half0).
  * w2 stats pass stashes w2 f32 in SBUF; quant reads the stash (no re-read).
"""

import os
import sys

for _p in ("/root/.axon_site/_ro/trn_rl_repo", "/opt/trn_rl_repo"):
    if os.path.isdir(_p) and _p not in sys.path:
        sys.path.append(_p)

from contextlib import ExitStack

import numpy as np

from concourse import bacc, bass, masks, mybir, tile
from concourse import bass_utils

F32 = mybir.dt.float32
BF16 = mybir.dt.bfloat16
AF = mybir.ActivationFunctionType
OP = mybir.AluOpType
AX = mybir.AxisListType

NCORES = 8
B, S, DIM, HID = 4, 2048, 2048, 8192
NTOK = B * S            # 8192
TOK = NTOK // NCORES    # 1024 tokens per core
KT = DIM // 128         # 16 k-tiles
HB = HID // 128         # 64 hid blocks
DB = DIM // 128         # 16 dim blocks
HBL = HID // NCORES // 128  # 8 hid blocks owned per core
CHUNKS = [1, 3, 4]      # w1 AG chunk sizes (hid blocks per core)
OFFS = [0, 1, 4]
NAG = len(CHUNKS)
MAGIC = 12582912.0      # 1.5 * 2**23: (v + MAGIC) - MAGIC == round-half-even(v)
EPS = 1e-6
W_NELEM = float(DIM * HID)

_cache = {}


def _build(n_cores=NCORES):
    nc = bacc.Bacc("TRN2", target_bir_lowering=False, debug=False, num_devices=n_cores)
    xT = nc.dram_tensor("xT", [DIM, TOK], F32, kind="ExternalInput")
    w1s = nc.dram_tensor("w1s", [DIM, HID // n_cores], F32, kind="ExternalInput")
    w2s = nc.dram_tensor("w2s", [HID // n_cores, DIM], F32, kind="ExternalInput")
    gpt = nc.dram_tensor("gpt", [128, KT], F32, kind="ExternalInput")
    outT = nc.dram_tensor("outT", [DIM, TOK], F32, kind="ExternalOutput")
    rg = [list(range(n_cores))]

    with tile.TileContext(nc) as tc, ExitStack() as ctx:
        misc = ctx.enter_context(tc.tile_pool(name="misc", bufs=1))
        ps_mm = ctx.enter_context(tc.tile_pool(name="ps_mm", bufs=2, space="PSUM"))
        ps_tr = ctx.enter_context(tc.tile_pool(name="ps_tr", bufs=2, space="PSUM"))
        ps_ss = ctx.enter_context(tc.tile_pool(name="ps_ss", bufs=1, space="PSUM"))
        dram = ctx.enter_context(tc.tile_pool(name="dram", bufs=1, space="DRAM"))

        ident = misc.tile([128, 128], F32)
        masks.make_identity(nc, ident[:])
        zero_col = misc.tile([128, 1], F32)
        nc.vector.memset(zero_col[:], 0.0)
        ones_row = misc.tile([1, 128], F32)
        nc.vector.memset(ones_row[:], 1.0)
        # rows shared across phases
        s1r = misc.tile([128, TOK], F32)
        s2r = misc.tile([128, TOK], F32)
        i2r = misc.tile([128, TOK], F32)
        invw = misc.tile([1, 2], F32)
        swb = misc.tile([128, 2], F32)

        def bcast_row(dst, src_row, n, off=0):
            """dst[128, off:off+n] = broadcast of src_row[1, n] via PE outer product."""
            for o in range(0, n, 512):
                w = min(512, n - o)
                ps = ps_mm.tile([128, 512], F32, tag="mm0")
                nc.tensor.matmul(ps[:, 0:w], ones_row[:], src_row[:, o:o + w],
                                 start=True, stop=True)
                nc.scalar.activation(dst[:, off + o:off + o + w], ps[:, 0:w], AF.Copy, bias=0.0)

        # DRAM scratch
        ar_in = dram.tile([2, 1], F32)
        ar_out = dram.tile([2, 1], F32, addr_space="Shared")
        t1_store = [dram.tile([CHUNKS[i], 128, KT * 128], BF16, name=f"t1s{i}")
                    for i in range(NAG)]
        t1_g = [dram.tile([n_cores, CHUNKS[i], 128, KT * 128], BF16, addr_space="Shared",
                          name=f"t1g{i}") for i in range(NAG)]
        t2_store = dram.tile([DB, 128, HBL * 128], BF16)
        t2_g = dram.tile([n_cores, DB, 128, HBL * 128], BF16, addr_space="Shared")
        r1_d = dram.tile([8, 128], F32)
        r2_d = dram.tile([8, 128], F32)

        with ExitStack() as s1:
            xq = s1.enter_context(tc.tile_pool(name="xq", bufs=1))
            nxT = xq.tile([128, KT * TOK], BF16)

            # ============ Phase A: W stats + X stats, fully interleaved =======
            with ExitStack() as sa:
                stash = sa.enter_context(tc.tile_pool(name="stash", bufs=1))
                wio = sa.enter_context(tc.tile_pool(name="wio", bufs=2))
                xio = sa.enter_context(tc.tile_pool(name="xio", bufs=2))
                scx = sa.enter_context(tc.tile_pool(name="scx", bufs=2))
                arow = sa.enter_context(tc.tile_pool(name="arow", bufs=1))
                rring = sa.enter_context(tc.tile_pool(name="rring", bufs=4))

                xg = stash.tile([128, KT * TOK], F32)       # x*gamma stash

                gam = arow.tile([128, KT], F32)
                nc.sync.dma_start(gam[:], gpt[:])
                ones_bf = arow.tile([128, 1], BF16)
                nc.vector.memset(ones_bf[:], 1.0)
                ones_f = arow.tile([128, 1], F32)
                nc.vector.memset(ones_f[:], 1.0)
                acc = arow.tile([128, TOK], F32)
                nc.vector.memset(acc[:], 0.0)
                S1c = arow.tile([128, KT], F32)
                S2c = arow.tile([128, HBL], F32)
                S12 = arow.tile([128, 2], F32)

                ss_ps0 = ps_ss.tile([1, 512], F32, tag="ss0")
                ss_ps1 = ps_ss.tile([1, 512], F32, tag="ss1")

                # ---- W stats first, split across queues; AR fires ~30us -----
                for kt in range(KT):
                    wt = wio.tile([128, HID // n_cores], F32, tag="w1t")
                    eng = nc.sync if kt % 2 == 0 else nc.scalar
                    eng.dma_start(wt[:], w1s[kt * 128:(kt + 1) * 128, :])
                    nc.vector.tensor_reduce(S1c[:, kt:kt + 1], wt[:], axis=AX.X, op=OP.add,
                                            apply_absolute_value=True)
                    if kt < HBL:
                        w2t = wio.tile([128, DIM], F32, tag="wq2")
                        nc.scalar.dma_start(w2t[:], w2s[kt * 128:(kt + 1) * 128, :])
                        nc.vector.tensor_reduce(S2c[:, kt:kt + 1], w2t[:], axis=AX.X,
                                                op=OP.add, apply_absolute_value=True)
                    # x loads early on the gpsimd queue (idle until the AR)
                    xt = xio.tile([128, TOK], F32, tag="xt")
                    nc.gpsimd.dma_start(xt[:], xT[kt * 128:(kt + 1) * 128, :])
                    x2 = scx.tile([128, TOK], BF16, tag="x2")
                    nc.scalar.activation(x2[:], xt[:], AF.Square, bias=zero_col[:])
                    nc.tensor.matmul(ss_ps0[:], ones_bf[:], x2[:, 0:512],
                                     start=(kt == 0), stop=(kt == KT - 1))
                    nc.tensor.matmul(ss_ps1[:], ones_bf[:], x2[:, 512:1024],
                                     start=(kt == 0), stop=(kt == KT - 1))
                    xgl = xg[:, kt * TOK:(kt + 1) * TOK]
                    nc.vector.tensor_scalar(xgl, xt[:], gam[:, kt:kt + 1], None, op0=OP.mult)
                    xga = scx.tile([128, TOK], F32, tag="xga")
                    nc.scalar.activation(xga[:], xgl, AF.Abs, bias=zero_col[:])
                    nc.vector.tensor_tensor(acc[:], acc[:], xga[:], op=OP.max)

                # |w| totals -> AllReduce (gpsimd; X-phase vector work overlaps it)
                nc.vector.tensor_reduce(S12[:, 0:1], S1c[:], axis=AX.X, op=OP.add)
                nc.vector.tensor_reduce(S12[:, 1:2], S2c[:], axis=AX.X, op=OP.add)
                tot_ps = ps_tr.tile([2, 1], F32, tag="tr")
                nc.tensor.matmul(tot_ps[:], S12[:], ones_f[:], start=True, stop=True)
                tot_sb = arow.tile([2, 1], F32)
                nc.vector.tensor_copy(tot_sb[:], tot_ps[:])
                nc.sync.dma_start(ar_in[:], tot_sb[:])
                nc.gpsimd.collective_compute(
                    "AllReduce", OP.add, replica_groups=rg, ins=[ar_in[:]], outs=[ar_out[:]])

                # ---- rows: rstd + absmax -> sx (runs during the AllReduce) ---
                v_row = rring.tile([1, TOK], F32, tag="row")
                nc.vector.tensor_scalar(v_row[:, 0:512], ss_ps0[:], 1.0 / DIM, EPS,
                                        op0=OP.mult, op1=OP.add)
                nc.vector.tensor_scalar(v_row[:, 512:1024], ss_ps1[:], 1.0 / DIM, EPS,
                                        op0=OP.mult, op1=OP.add)
                sq_row = rring.tile([1, TOK], F32, tag="row")
                nc.scalar.activation(sq_row[:], v_row[:], AF.Sqrt, bias=zero_col[0:1, :])
                rscr = arow.tile([1, TOK], F32)
                rstd_row = rring.tile([1, TOK], F32, tag="row")
                nc.vector.reciprocal_approx_accurate(rstd_row[:], sq_row[:], rscr[:])

                m0 = arow.tile([128, 8], F32)
                for c in range(8):
                    pt = ps_tr.tile([128, 128], F32, tag="tr")
                    nc.tensor.transpose(pt[:], acc[:, c * 128:(c + 1) * 128], ident[:])
                    nc.vector.tensor_reduce(m0[:, c:c + 1], pt[:], axis=AX.X, op=OP.max)
                nc.sync.dma_start(r1_d[:].rearrange("c p -> p c"), m0[:])
                m0row = rring.tile([1, TOK], F32, tag="row")
                nc.sync.dma_start(m0row[:], r1_d[:].rearrange("c p -> (c p)")[None, :])
                nc.vector.tensor_tensor(m0row[:], m0row[:], rstd_row[:], op=OP.mult)
                nc.vector.tensor_scalar(m0row[:], m0row[:], 1e-5, None, op0=OP.max)
                sx_row0 = rring.tile([1, TOK], F32, tag="row")
                nc.vector.reciprocal_approx_accurate(sx_row0[:], m0row[:], rscr[:])
                nc.vector.tensor_scalar(sx_row0[:], sx_row0[:], 127.0, None, op0=OP.mult)
                inv_sx = rring.tile([1, TOK], F32, tag="row")
                nc.vector.reciprocal_approx_accurate(inv_sx[:], sx_row0[:], rscr[:])
                nc.vector.tensor_tensor(rstd_row[:], rstd_row[:], sx_row0[:], op=OP.mult)
                rsx = arow.tile([128, TOK], F32)
                bcast_row(rsx, rstd_row, TOK)

                # ---- AllReduce result -> scales (vector stalls here ~80us) ---
                tot2 = rring.tile([1, TOK], F32, tag="row")
                nc.sync.dma_start(tot2[:, 0:2], ar_out[:].rearrange("a b -> b a"))
                nc.vector.tensor_scalar(invw[:], tot2[:, 0:2], 1.0 / W_NELEM, 1e-5,
                                        op0=OP.mult, op1=OP.max)
                sw = rring.tile([1, TOK], F32, tag="row")
                nc.vector.reciprocal(sw[:, 0:2], invw[:])
                ps_b = ps_tr.tile([128, 2], F32, tag="tr")
                nc.tensor.matmul(ps_b[:], ones_row[:], sw[:, 0:2], start=True, stop=True)
                nc.scalar.activation(swb[:], ps_b[:], AF.Copy, bias=0.0)

                # ---- WQ w1 chunks: quant (vector) + store + AG (gpsimd) ------
                for ci in range(NAG):
                    CH = CHUNKS[ci]
                    CW = CH * 128
                    for kt in range(KT):
                        wq = wio.tile([128, 4 * 128], F32, tag="wq")
                        nc.sync.dma_start(wq[:, 0:CW], w1s[kt * 128:(kt + 1) * 128,
                                                           OFFS[ci] * 128:OFFS[ci] * 128 + CW])
                        nc.vector.tensor_scalar(wq[:, 0:CW], wq[:, 0:CW], swb[:, 0:1], -1.0,
                                                op0=OP.mult, op1=OP.max)
                        nc.vector.tensor_scalar(wq[:, 0:CW], wq[:, 0:CW], 1.0, MAGIC,
                                                op0=OP.min, op1=OP.add)
                        q = wio.tile([128, 4 * 128], BF16, tag="q")
                        nc.vector.tensor_scalar(q[:, 0:CW], wq[:, 0:CW], MAGIC, None,
                                                op0=OP.subtract)
                        nc.gpsimd.dma_start(
                            t1_store[ci][:, :, kt * 128:(kt + 1) * 128].rearrange(
                                "b k j -> k b j"),
                            q[:, 0:CW].rearrange("k (b j) -> k b j", b=CH))
                    nc.gpsimd.collective_compute(
                        "AllGather", OP.bypass, replica_groups=rg,
                        ins=[t1_store[ci][:]], outs=[t1_g[ci][:]])

                # quantize: n_xT = round(xg * rstd*sx)  (bf16 ints; before MM1)
                for kt in range(KT):
                    t = scx.tile([128, TOK], F32, tag="xq")
                    nc.vector.tensor_tensor(t[:], xg[:, kt * TOK:(kt + 1) * TOK], rsx[:],
                                            op=OP.mult)
                    nc.vector.tensor_scalar(nxT[:, kt * TOK:(kt + 1) * TOK], t[:], MAGIC, MAGIC,
                                            op0=OP.add, op1=OP.subtract)
                s1_row0 = rring.tile([1, TOK], F32, tag="row")
                nc.vector.tensor_scalar(s1_row0[:], inv_sx[:], invw[:, 0:1], None, op0=OP.mult)
                bcast_row(s1r, s1_row0, TOK)

                # ---- w2 quant ENTIRELY on gpsimd, dep-pinned after AG3 so the
                # scheduler cannot hoist its (long) AllGather ahead of w1's ----
                dumm = arow.tile([128, 2], F32)
                nc.gpsimd.dma_start(
                    dumm[:], t1_g[NAG - 1][0, 0, :, 0:2])
                sw2p = arow.tile([128, 1], F32)
                nc.gpsimd.tensor_scalar(sw2p[:], dumm[:, 0:1], 0.0, None, op0=OP.mult)
                nc.gpsimd.tensor_tensor(sw2p[:], sw2p[:], swb[:, 1:2], op=OP.add)
                for ht in range(HBL):
                    wq2 = wio.tile([128, DIM], F32, tag="wq2")
                    nc.gpsimd.dma_start(wq2[:], w2s[ht * 128:(ht + 1) * 128, :])
                    nc.gpsimd.tensor_scalar(wq2[:], wq2[:], sw2p[:, 0:1], -1.0,
                                            op0=OP.mult, op1=OP.max)
                    nc.gpsimd.tensor_scalar(wq2[:], wq2[:], 1.0, MAGIC, op0=OP.min, op1=OP.add)
                    q2 = wio.tile([128, DIM], BF16, tag="q2")
                    nc.gpsimd.tensor_scalar(q2[:], wq2[:], MAGIC, None, op0=OP.subtract)
                    nc.gpsimd.dma_start(
                        t2_store[:, :, ht * 128:(ht + 1) * 128].rearrange("d k j -> k d j"),
                        q2[:].rearrange("k (d j) -> k d j", d=DB))
                nc.gpsimd.collective_compute(
                    "AllGather", OP.bypass, replica_groups=rg, ins=[t2_store[:]], outs=[t2_g[:]])

            # ============ Phase MM1 + gelu + absmax (h stays in SBUF) =========
            with ExitStack() as sb:
                hp = sb.enter_context(tc.tile_pool(name="hp", bufs=1))
                h0 = hp.tile([128, HB * 512], BF16)
                h1 = hp.tile([128, HB * 512], BF16)

                with ExitStack() as sb1:
                    w1st = sb1.enter_context(tc.tile_pool(name="w1st", bufs=2))
                    scr = sb1.enter_context(tc.tile_pool(name="scr", bufs=2))
                    am = sb1.enter_context(tc.tile_pool(name="am", bufs=1))
                    rr2 = sb1.enter_context(tc.tile_pool(name="rr2", bufs=2))

                    acc2 = am.tile([128, TOK], F32)
                    nc.vector.memset(acc2[:], 0.0)
                    for ci in range(NAG):
                        for r in range(n_cores):
                            for bi in range(CHUNKS[ci]):
                                ghb = r * HBL + OFFS[ci] + bi
                                wb = w1st.tile([128, KT * 128], BF16, tag="wb")
                                nc.sync.dma_start(wb[:], t1_g[ci][r, bi])
                                ps0 = ps_mm.tile([128, 512], F32, tag="mm0")
                                ps1 = ps_mm.tile([128, 512], F32, tag="mm1")
                                for kt in range(KT):
                                    st, sp = (kt == 0), (kt == KT - 1)
                                    nc.tensor.matmul(ps0[:], wb[:, kt * 128:(kt + 1) * 128],
                                                     nxT[:, kt * TOK:kt * TOK + 512],
                                                     start=st, stop=sp)
                                    nc.tensor.matmul(ps1[:], wb[:, kt * 128:(kt + 1) * 128],
                                                     nxT[:, kt * TOK + 512:kt * TOK + 1024],
                                                     start=st, stop=sp)
                                for th, ps, htile in ((0, ps0, h0), (1, ps1, h1)):
                                    sl = slice(th * 512, th * 512 + 512)
                                    hsl = htile[:, ghb * 512:(ghb + 1) * 512]
                                    hs = scr.tile([128, 512], F32, tag="hs")
                                    nc.vector.tensor_tensor(hs[:], ps[:], s1r[:, sl], op=OP.mult)
                                    nc.scalar.activation(hsl, hs[:], AF.Gelu, bias=zero_col[:])
                                    ga = scr.tile([128, 512], F32, tag="ga")
                                    nc.scalar.activation(ga[:], hsl, AF.Abs, bias=zero_col[:])
                                    nc.vector.tensor_tensor(acc2[:, sl], acc2[:, sl], ga[:],
                                                            op=OP.max)

                    # scale2 rows
                    m2 = am.tile([128, 8], F32)
                    for c in range(8):
                        pt = ps_tr.tile([128, 128], F32, tag="tr")
                        nc.tensor.transpose(pt[:], acc2[:, c * 128:(c + 1) * 128], ident[:])
                        nc.vector.tensor_reduce(m2[:, c:c + 1], pt[:], axis=AX.X, op=OP.max)
                    nc.sync.dma_start(r2_d[:].rearrange("c p -> p c"), m2[:])
                    m2row = rr2.tile([1, TOK], F32, tag="row")
                    nc.sync.dma_start(m2row[:], r2_d[:].rearrange("c p -> (c p)")[None, :])
                    nc.vector.tensor_scalar(m2row[:], m2row[:], 1e-5, None, op0=OP.max)
                    rscr2 = am.tile([1, TOK], F32)
                    s2_row0 = rr2.tile([1, TOK], F32, tag="row")
                    nc.vector.reciprocal_approx_accurate(s2_row0[:], m2row[:], rscr2[:])
                    nc.vector.tensor_scalar(s2_row0[:], s2_row0[:], 127.0, None, op0=OP.mult)
                    i2_row0 = rr2.tile([1, TOK], F32, tag="row")
                    nc.vector.reciprocal_approx_accurate(i2_row0[:], s2_row0[:], rscr2[:])
                    nc.vector.tensor_scalar(i2_row0[:], i2_row0[:], invw[:, 1:2], None,
                                            op0=OP.mult)
                    bcast_row(s2r, s2_row0, TOK)
                    bcast_row(i2r, i2_row0, TOK)

                # ============ Q2 (in-place) + MM2, per token-half ==============
                with ExitStack() as sb2:
                    w2st = sb2.enter_context(tc.tile_pool(name="w2st", bufs=2))
                    hbk = sb2.enter_context(tc.tile_pool(name="hbk", bufs=3))
                    hio = sb2.enter_context(tc.tile_pool(name="hio", bufs=3))
                    nr2 = n_cores // 2
                    for th, htile in ((0, h0), (1, h1)):
                        to = th * 512
                        st2 = s2r[:, to:to + 512]
                        for kg in range(HB):
                            hsl = htile[:, kg * 512:(kg + 1) * 512]
                            t2s = hbk.tile([128, 512], F32, tag="t2")
                            nc.vector.tensor_tensor(t2s[:], hsl, st2, op=OP.mult)
                            nc.vector.tensor_scalar(hsl, t2s[:], MAGIC, MAGIC,
                                                    op0=OP.add, op1=OP.subtract)
                        for d in range(DB):
                            ps = ps_mm.tile([128, 512], F32, tag=f"mm{th}")
                            for half in range(2):
                                wv = w2st.tile([128, nr2 * HBL * 128], BF16, tag="w2")
                                nc.sync.dma_start(
                                    wv[:].rearrange("k (r f) -> k r f", r=nr2),
                                    t2_g[half * nr2:(half + 1) * nr2, d].rearrange(
                                        "r k f -> k r f"))
                                for k2 in range(HB // 2):
                                    kg = half * (HB // 2) + k2
                                    nc.tensor.matmul(ps[:], wv[:, k2 * 128:(k2 + 1) * 128],
                                                     htile[:, kg * 512:(kg + 1) * 512],
                                                     start=(kg == 0), stop=(kg == HB - 1))
                            ot = hio.tile([128, 512], F32, tag="ot")
                            nc.vector.tensor_tensor(ot[:], ps[:], i2r[:, to:to + 512], op=OP.mult)
                            nc.sync.dma_start(outT[d * 128:(d + 1) * 128, to:to + 512], ot[:])

    nc.compile()
    return nc


def _get_nc():
    if "nc" not in _cache:
        _cache["nc"] = _build()
    return _cache["nc"]


def _prep_inputs(x, w1, w2, gamma):
    x2d = np.ascontiguousarray(np.asarray(x, dtype=np.float32).reshape(NTOK, DIM))
    w1 = np.asarray(w1, dtype=np.float32)
    w2 = np.asarray(w2, dtype=np.float32)
    gamma = np.asarray(gamma, dtype=np.float32)
    w1T = np.ascontiguousarray(w1.T)          # [DIM, HID]
    w2T = np.ascontiguousarray(w2.T)          # [HID, DIM]
    gpt = np.ascontiguousarray(gamma.reshape(KT, 128).T)
    hs = HID // NCORES
    in_maps = []
    for c in range(NCORES):
        in_maps.append({
            "xT": np.ascontiguousarray(x2d[c * TOK:(c + 1) * TOK, :].T),
            "w1s": np.ascontiguousarray(w1T[:, c * hs:(c + 1) * hs]),
            "w2s": np.ascontiguousarray(w2T[c * hs:(c + 1) * hs, :]),
            "gpt": gpt,
        })
    return in_maps


def _run(in_maps, trace=False, **kw):
    nc = _get_nc()
    return bass_utils.run_bass_kernel_spmd(
        nc, in_maps, core_ids=list(range(NCORES)), trace=trace, **kw)


def kernel(x, w1, w2, gamma):
    in_maps = _prep_inputs(x, w1, w2, gamma)
    res = _run(in_maps, trace=False)
    out = np.empty((NTOK, DIM), dtype=np.float32)
    for c in range(NCORES):
        out[c * TOK:(c + 1) * TOK, :] = res.results[c]["outT"].T
    return out.reshape(B, S, DIM)



# revision 7
# speedup vs baseline: 1.2773x; 1.2773x over previous
"""BitMLP (BitNet-style MLP) Trainium2 kernel, 8-way data-parallel over tokens.

reference semantics:
  h   = act_quant(rms_norm(x, gamma)) @ w1q.T   (w1q = per-tensor ternary quant)
  out = act_quant(gelu_exact(h)) @ w2q.T

Key facts exploited:
  * act_quant produces n/scale with n an integer in [-127, 127]  -> n is exact in bf16
  * weight quant produces t*inv_w with t ternary in {-1, 0, 1}   -> t is exact in bf16
  * therefore both matmuls are exact integer accumulations computed in bf16 at
    full TensorE rate; per-token/per-tensor scales are applied afterwards.

Sharding (8 cores on one chip):
  * tokens (4*2048 = 8192) split 1024/core; each core computes its tokens' output
  * weight quantization is cooperative: core c quantizes 1/8 of w1 and w2,
    the per-tensor mean(|w|) is combined with a scalar AllReduce, and the
    ternary bf16 weights are AllGathered.

v3 schedule (from trace analysis of v2: a 725us gpsimd w2-quant wall serialized
the whole kernel, and MM1 didn't start until 1.1ms):
  * w2 quant runs on VECTOR (~26us), right after the x-quant pass; only the
    AllGather ORDER is pinned (tiny gpsimd data-dep chain through t1_g[2]) so
    the 134us w2 AllGather cannot be scheduled ahead of w1's chunked AGs.
  * MM1 is token-half-outer (A = tokens 0:512 for all 64 hid blocks, then B):
    h(A)'s absmax + requant (Q2) hide under MM1(B); Q2(B) hides under MM2(A).
    No PE stall at the MM1->MM2 boundary.
  * MM2 is d-outer within each token half (w2 ternary read once per half).
  * Phase A is queue-balanced: w1 stats on sync, w2 stats + x on scalar,
    collectives + row round-trips + w2 loads + out stores on gpsimd; the
    AllReduce triggers ~30us in, AG chunks [2,3,3] pipeline under MM1(A).
  * No x*gamma stash: x is re-read from DRAM for the quant pass, freeing SBUF
    so nxT + both h halves are resident (h1's pool opens after phase A's
    scratch pool closes - no WAR stall into MM1(B)).
"""

import os
import sys

for _p in ("/root/.axon_site/_ro/trn_rl_repo", "/opt/trn_rl_repo"):
    if os.path.isdir(_p) and _p not in sys.path:
        sys.path.append(_p)

from contextlib import ExitStack

import numpy as np

from concourse import bacc, bass, masks, mybir, tile
from concourse import bass_utils

F32 = mybir.dt.float32
BF16 = mybir.dt.bfloat16
AF = mybir.ActivationFunctionType
OP = mybir.AluOpType
AX = mybir.AxisListType

NCORES = 8
B, S, DIM, HID = 4, 2048, 2048, 8192
NTOK = B * S            # 8192
TOK = NTOK // NCORES    # 1024 tokens per core
HTOK = TOK // 2         # 512: token half processed as one PE pass
KT = DIM // 128         # 16 k-tiles
HB = HID // 128         # 64 hid blocks
DB = DIM // 128         # 16 dim blocks
HBL = HID // NCORES // 128  # 8 hid blocks owned per core
CHUNKS = [2, 3, 3]      # w1 AG chunk sizes (hid blocks per core)
OFFS = [0, 2, 5]
NAG = len(CHUNKS)
MAGIC = 12582912.0      # 1.5 * 2**23: (v + MAGIC) - MAGIC == round-half-even(v)
EPS = 1e-6
W_NELEM = float(DIM * HID)

_cache = {}


def _build(n_cores=NCORES):
    nc = bacc.Bacc("TRN2", target_bir_lowering=False, debug=False, num_devices=n_cores)
    xT = nc.dram_tensor("xT", [DIM, TOK], F32, kind="ExternalInput")
    w1s = nc.dram_tensor("w1s", [DIM, HID // n_cores], F32, kind="ExternalInput")
    w2s = nc.dram_tensor("w2s", [HID // n_cores, DIM], F32, kind="ExternalInput")
    gpt = nc.dram_tensor("gpt", [128, KT], F32, kind="ExternalInput")
    outT = nc.dram_tensor("outT", [DIM, TOK], F32, kind="ExternalOutput")
    rg = [list(range(n_cores))]

    with tile.TileContext(nc) as tc, ExitStack() as ctx:
        misc = ctx.enter_context(tc.tile_pool(name="misc", bufs=1))
        rowp = ctx.enter_context(tc.tile_pool(name="rowp", bufs=2))
        xq = ctx.enter_context(tc.tile_pool(name="xq", bufs=1))
        hp0 = ctx.enter_context(tc.tile_pool(name="hp0", bufs=1))
        pw = ctx.enter_context(tc.tile_pool(name="pw", bufs=3))
        psc = ctx.enter_context(tc.tile_pool(name="psc", bufs=2))
        pmm2 = ctx.enter_context(tc.tile_pool(name="pmm2", bufs=2))
        ps_mm = ctx.enter_context(tc.tile_pool(name="ps_mm", bufs=4, space="PSUM"))
        ps_tr = ctx.enter_context(tc.tile_pool(name="ps_tr", bufs=2, space="PSUM"))
        ps_ss = ctx.enter_context(tc.tile_pool(name="ps_ss", bufs=1, space="PSUM"))
        dram = ctx.enter_context(tc.tile_pool(name="dram", bufs=1, space="DRAM"))

        ident = misc.tile([128, 128], F32)
        masks.make_identity(nc, ident[:])
        zero_col = misc.tile([128, 1], F32)
        nc.vector.memset(zero_col[:], 0.0)
        ones_row = misc.tile([1, 128], F32)
        nc.vector.memset(ones_row[:], 1.0)
        ones_bf = misc.tile([128, 1], BF16)
        nc.vector.memset(ones_bf[:], 1.0)
        ones_f = misc.tile([128, 1], F32)
        nc.vector.memset(ones_f[:], 1.0)
        # persistent scale rows / broadcast tiles
        s1r = misc.tile([128, TOK], F32)        # (invw1 * inv_sx) per token
        s2r = misc.tile([128, TOK], F32)        # s2 per token, both halves
        i2r = misc.tile([128, TOK], F32)        # invw2 * inv_s2 per token
        invw = misc.tile([1, 2], F32)
        swb = misc.tile([128, 2], F32)
        gam = misc.tile([128, KT], F32)
        acc = misc.tile([128, TOK], F32)        # absmax accumulator (reused per phase)
        S1c = misc.tile([128, KT], F32)
        S2c = misc.tile([128, KT], F32)
        S12 = misc.tile([128, 2], F32)
        tot_sb = misc.tile([2, 1], F32)
        m0t = misc.tile([128, 8], F32)
        m2t = misc.tile([128, 4], F32)

        def bcast_row(dst, src_row, n, off=0):
            """dst[128, off:off+n] = broadcast of src_row[1, n] via PE outer product."""
            for o in range(0, n, 512):
                w = min(512, n - o)
                ps = ps_mm.tile([128, 512], F32, tag="mm")
                nc.tensor.matmul(ps[:, 0:w], ones_row[:], src_row[:, o:o + w],
                                 start=True, stop=True)
                nc.scalar.activation(dst[:, off + o:off + o + w], ps[:, 0:w], AF.Copy, bias=0.0)

        # DRAM scratch
        ar_in = dram.tile([2, 1], F32)
        ar_out = dram.tile([2, 1], F32, addr_space="Shared")
        t1_store = [dram.tile([CHUNKS[i], 128, KT * 128], BF16, name=f"t1s{i}")
                    for i in range(NAG)]
        t1_g = [dram.tile([n_cores, CHUNKS[i], 128, KT * 128], BF16, addr_space="Shared",
                          name=f"t1g{i}") for i in range(NAG)]
        t2_store = dram.tile([DB, 128, HBL * 128], BF16)
        t2_g = dram.tile([n_cores, DB, 128, HBL * 128], BF16, addr_space="Shared")
        r1_d = dram.tile([8, 128], F32)
        r2_d = [dram.tile([4, 128], F32, name=f"r2d{t}") for t in range(2)]

        nc.sync.dma_start(gam[:], gpt[:])

        nxT = xq.tile([128, KT * TOK], BF16)
        h0 = hp0.tile([128, HB * HTOK], BF16)

        with ExitStack() as sa:
            big = sa.enter_context(tc.tile_pool(name="big", bufs=3))
            scx2 = sa.enter_context(tc.tile_pool(name="scx2", bufs=1))
            xgm = sa.enter_context(tc.tile_pool(name="xgm", bufs=1))
            xgap = sa.enter_context(tc.tile_pool(name="xgap", bufs=1))
            wio = sa.enter_context(tc.tile_pool(name="wio", bufs=4))
            qio = sa.enter_context(tc.tile_pool(name="qio", bufs=1))
            wio2 = sa.enter_context(tc.tile_pool(name="wio2", bufs=2))
            qio2 = sa.enter_context(tc.tile_pool(name="qio2", bufs=2))
            rring = sa.enter_context(tc.tile_pool(name="rring", bufs=3))
            sax = sa.enter_context(tc.tile_pool(name="sax", bufs=1))

            rsx = sax.tile([128, TOK], F32)
            inv_sx = sax.tile([1, TOK], F32)
            rscr = sax.tile([1, TOK], F32)
            xga = xgap.tile([128, TOK], F32)

            nc.vector.memset(acc[:], 0.0)

            # ---- |w1| stats: loads on sync, reduces on vector ---------------
            for kt in range(KT):
                wt = big.tile([128, TOK], F32, tag="big")
                nc.sync.dma_start(wt[:], w1s[kt * 128:(kt + 1) * 128, :])
                nc.vector.tensor_reduce(S1c[:, kt:kt + 1], wt[:], axis=AX.X, op=OP.add,
                                        apply_absolute_value=True)
            # ---- |w2| stats: loads on scalar (half-row tiles) ---------------
            for ht in range(HBL):
                for hf in range(2):
                    w2t = big.tile([128, TOK], F32, tag="big")
                    nc.scalar.dma_start(w2t[:], w2s[ht * 128:(ht + 1) * 128,
                                                    hf * 1024:(hf + 1) * 1024])
                    nc.vector.tensor_reduce(S2c[:, 2 * ht + hf:2 * ht + hf + 1], w2t[:],
                                            axis=AX.X, op=OP.add,
                                            apply_absolute_value=True)
            nc.vector.tensor_reduce(S12[:, 0:1], S1c[:], axis=AX.X, op=OP.add)
            nc.vector.tensor_reduce(S12[:, 1:2], S2c[:], axis=AX.X, op=OP.add)
            tot_ps = ps_tr.tile([2, 1], F32, tag="tr")
            nc.tensor.matmul(tot_ps[:], S12[:], ones_f[:], start=True, stop=True)
            nc.vector.tensor_copy(tot_sb[:], tot_ps[:])
            nc.gpsimd.dma_start(ar_in[:], tot_sb[:])
            nc.gpsimd.collective_compute(
                "AllReduce", OP.add, replica_groups=rg, ins=[ar_in[:]], outs=[ar_out[:]])

            # ---- X stats pass: loads + square/abs on scalar, max on vector --
            ss_ps0 = ps_ss.tile([1, 512], F32, tag="ss0")
            ss_ps1 = ps_ss.tile([1, 512], F32, tag="ss1")
            for kt in range(KT):
                xt = big.tile([128, TOK], F32, tag="big")
                nc.scalar.dma_start(xt[:], xT[kt * 128:(kt + 1) * 128, :])
                x2 = scx2.tile([128, TOK], BF16, tag="x2")
                nc.scalar.activation(x2[:], xt[:], AF.Square, bias=zero_col[:])
                nc.tensor.matmul(ss_ps0[:], ones_bf[:], x2[:, 0:512],
                                 start=(kt == 0), stop=(kt == KT - 1))
                nc.tensor.matmul(ss_ps1[:], ones_bf[:], x2[:, 512:1024],
                                 start=(kt == 0), stop=(kt == KT - 1))
                xg = xgm.tile([128, TOK], F32, tag="xgm")
                nc.vector.tensor_scalar(xg[:], xt[:], gam[:, kt:kt + 1], None, op0=OP.mult)
                nc.scalar.activation(xga[:], xg[:], AF.Abs, bias=zero_col[:])
                nc.vector.tensor_tensor(acc[:], acc[:], xga[:], op=OP.max)

            # ---- token rows: rstd + absmax -> sx, rsx -----------------------
            v_row = rring.tile([1, TOK], F32, tag="row")
            nc.vector.tensor_scalar(v_row[:, 0:512], ss_ps0[:], 1.0 / DIM, EPS,
                                    op0=OP.mult, op1=OP.add)
            nc.vector.tensor_scalar(v_row[:, 512:1024], ss_ps1[:], 1.0 / DIM, EPS,
                                    op0=OP.mult, op1=OP.add)
            sq_row = rring.tile([1, TOK], F32, tag="row")
            nc.scalar.activation(sq_row[:], v_row[:], AF.Sqrt, bias=zero_col[0:1, :])
            rstd_row = rring.tile([1, TOK], F32, tag="row")
            nc.vector.reciprocal_approx_accurate(rstd_row[:], sq_row[:], rscr[:])

            for c in range(8):
                pt = ps_tr.tile([128, 128], F32, tag="tr")
                nc.tensor.transpose(pt[:], acc[:, c * 128:(c + 1) * 128], ident[:])
                nc.vector.tensor_reduce(m0t[:, c:c + 1], pt[:], axis=AX.X, op=OP.max)
            nc.gpsimd.dma_start(r1_d[:].rearrange("c p -> p c"), m0t[:])
            m0row = rring.tile([1, TOK], F32, tag="row")
            nc.gpsimd.dma_start(m0row[:], r1_d[:].rearrange("c p -> (c p)")[None, :])
            nc.vector.tensor_tensor(m0row[:], m0row[:], rstd_row[:], op=OP.mult)
            nc.vector.tensor_scalar(m0row[:], m0row[:], 1e-5, None, op0=OP.max)
            sx_row = rring.tile([1, TOK], F32, tag="row")
            nc.vector.reciprocal_approx_accurate(sx_row[:], m0row[:], rscr[:])
            nc.vector.tensor_scalar(sx_row[:], sx_row[:], 127.0, None, op0=OP.mult)
            nc.vector.reciprocal_approx_accurate(inv_sx[:], sx_row[:], rscr[:])
            nc.vector.tensor_tensor(rstd_row[:], rstd_row[:], sx_row[:], op=OP.mult)
            bcast_row(rsx, rstd_row, TOK)

            # ---- AllReduce result -> weight scales --------------------------
            tot2 = rring.tile([1, TOK], F32, tag="row")
            nc.sync.dma_start(tot2[:, 0:2], ar_out[:].rearrange("a b -> b a"))
            nc.vector.tensor_scalar(invw[:], tot2[:, 0:2], 1.0 / W_NELEM, 1e-5,
                                    op0=OP.mult, op1=OP.max)
            sw = rring.tile([1, TOK], F32, tag="row")
            nc.vector.reciprocal(sw[:, 0:2], invw[:])
            ps_b = ps_tr.tile([128, 2], F32, tag="tr")
            nc.tensor.matmul(ps_b[:], ones_row[:], sw[:, 0:2], start=True, stop=True)
            nc.scalar.activation(swb[:], ps_b[:], AF.Copy, bias=0.0)

            # ---- WQ w1 chunks: re-read (sync) + quant (vector) + AG ---------
            for ci in range(NAG):
                CH = CHUNKS[ci]
                CW = CH * 128
                for kt in range(KT):
                    wq = wio.tile([128, 3 * 128], F32, tag="wq")
                    nc.sync.dma_start(wq[:, 0:CW], w1s[kt * 128:(kt + 1) * 128,
                                                       OFFS[ci] * 128:OFFS[ci] * 128 + CW])
                    nc.vector.tensor_scalar(wq[:, 0:CW], wq[:, 0:CW], swb[:, 0:1], -1.0,
                                            op0=OP.mult, op1=OP.max)
                    nc.vector.tensor_scalar(wq[:, 0:CW], wq[:, 0:CW], 1.0, MAGIC,
                                            op0=OP.min, op1=OP.add)
                    q = qio.tile([128, 3 * 128], BF16, tag="q")
                    nc.vector.tensor_scalar(q[:, 0:CW], wq[:, 0:CW], MAGIC, None,
                                            op0=OP.subtract)
                    nc.gpsimd.dma_start(
                        t1_store[ci][:, :, kt * 128:(kt + 1) * 128].rearrange(
                            "b k j -> k b j"),
                        q[:, 0:CW].rearrange("k (b j) -> k b j", b=CH))
                nc.gpsimd.collective_compute(
                    "AllGather", OP.bypass, replica_groups=rg,
                    ins=[t1_store[ci][:]], outs=[t1_g[ci][:]])

            # ---- quantize x: re-read (sync), n_xT = round(x*gam*rsx) --------
            s1_row = rring.tile([1, TOK], F32, tag="row")
            for kt in range(KT):
                xr = big.tile([128, TOK], F32, tag="big")
                nc.sync.dma_start(xr[:], xT[kt * 128:(kt + 1) * 128, :])
                t = xgm.tile([128, TOK], F32, tag="xgm")
                nc.vector.tensor_scalar(t[:], xr[:], gam[:, kt:kt + 1], None, op0=OP.mult)
                nc.vector.tensor_tensor(t[:], t[:], rsx[:], op=OP.mult)
                nc.vector.tensor_scalar(nxT[:, kt * TOK:(kt + 1) * TOK], t[:], MAGIC, MAGIC,
                                        op0=OP.add, op1=OP.subtract)
            nc.vector.tensor_scalar(s1_row[:], inv_sx[:], invw[:, 0:1], None, op0=OP.mult)
            bcast_row(s1r, s1_row, TOK)
            # reset absmax accumulator for the h phase (accA | accB halves)
            nc.vector.memset(acc[:], 0.0)

            # ---- w2 quant on vector; loads + stores on gpsimd ---------------
            for ht in range(HBL):
                for hf in range(2):
                    w2l = wio2.tile([128, TOK], F32, tag="w2")
                    nc.gpsimd.dma_start(w2l[:], w2s[ht * 128:(ht + 1) * 128,
                                                    hf * 1024:(hf + 1) * 1024])
                    nc.vector.tensor_scalar(w2l[:], w2l[:], swb[:, 1:2], -1.0,
                                            op0=OP.mult, op1=OP.max)
                    nc.vector.tensor_scalar(w2l[:], w2l[:], 1.0, MAGIC,
                                            op0=OP.min, op1=OP.add)
                    q2 = qio2.tile([128, TOK], BF16, tag="q2")
                    nc.vector.tensor_scalar(q2[:], w2l[:], MAGIC, None, op0=OP.subtract)
                    d0 = hf * 8
                    nc.gpsimd.dma_start(
                        t2_store[d0:d0 + 8, :, ht * 128:(ht + 1) * 128].rearrange(
                            "d k j -> k d j"),
                        q2[:].rearrange("k (d j) -> k d j", d=8))
            # pin: last write into t2_store is data-dependent on t1_g[2] (AG2
            # output), so the w2 AllGather cannot be scheduled before w1's AGs.
            pin_t = misc.tile([2, 1], BF16)
            pin_d = misc.tile([2, 1], BF16)
            pin_z = misc.tile([2, 1], BF16)
            nc.gpsimd.dma_start(pin_t[:], t2_store[0, 0:2, 0:1])
            nc.gpsimd.dma_start(pin_d[:], t1_g[NAG - 1][0, 0, 0:2, 0:1])
            nc.gpsimd.tensor_scalar(pin_z[:], pin_d[:], 0.0, None, op0=OP.mult)
            nc.gpsimd.tensor_tensor(pin_z[:], pin_z[:], pin_t[:], op=OP.add)
            nc.gpsimd.dma_start(t2_store[0, 0:2, 0:1], pin_z[:])
            nc.gpsimd.collective_compute(
                "AllGather", OP.bypass, replica_groups=rg, ins=[t2_store[:]], outs=[t2_g[:]])

            # ============ MM1, token half A (all 64 hid blocks) ==============
            blocks = [(ci, r, bi) for ci in range(NAG) for r in range(n_cores)
                      for bi in range(CHUNKS[ci])]

            def mm1_block(ci, r, bi, th, htile):
                ghb = r * HBL + OFFS[ci] + bi
                to = th * 512
                wbt = [pw.tile([128, 1024], BF16, tag="wb", name=f"wb{_f}")
                       for _f in range(2)]
                for hf in range(2):
                    nc.sync.dma_start(wbt[hf][:], t1_g[ci][r, bi][:, hf * 1024:(hf + 1) * 1024])
                ps = ps_mm.tile([128, 512], F32, tag="mm")
                for kt in range(KT):
                    wsl = wbt[kt // 8][:, (kt % 8) * 128:(kt % 8 + 1) * 128]
                    nc.tensor.matmul(ps[:], wsl, nxT[:, kt * TOK + to:kt * TOK + to + 512],
                                     start=(kt == 0), stop=(kt == KT - 1))
                hs = psc.tile([128, 512], F32, tag="hs")
                nc.vector.tensor_tensor(hs[:], ps[:], s1r[:, to:to + 512], op=OP.mult)
                hsl = htile[:, ghb * 512:(ghb + 1) * 512]
                nc.scalar.activation(hsl, hs[:], AF.Gelu, bias=zero_col[:])
                ga = psc.tile([128, 512], BF16, tag="ga")
                nc.scalar.activation(ga[:], hsl, AF.Abs, bias=zero_col[:])
                nc.vector.tensor_tensor(acc[:, to:to + 512], acc[:, to:to + 512], ga[:],
                                        op=OP.max)

            for (ci, r, bi) in blocks:
                mm1_block(ci, r, bi, 0, h0)

        # ---- phase A scratch pool closed; h1 lives in its space -------------
        with ExitStack() as sb:
            hp1 = sb.enter_context(tc.tile_pool(name="hp1", bufs=1))
            h1 = hp1.tile([128, HB * HTOK], BF16)

            def half_rows(th):
                """acc[:, th*512:+512] absmax -> s2r/i2r[:, th-half] rows."""
                to = th * 512
                for c in range(4):
                    pt = ps_tr.tile([128, 128], F32, tag="tr")
                    nc.tensor.transpose(pt[:], acc[:, to + c * 128:to + (c + 1) * 128],
                                        ident[:])
                    nc.vector.tensor_reduce(m2t[:, c:c + 1], pt[:], axis=AX.X, op=OP.max)
                nc.gpsimd.dma_start(r2_d[th][:].rearrange("c p -> p c"), m2t[:])
                m2row = rowp.tile([1, 512], F32, tag="lrow")
                nc.gpsimd.dma_start(m2row[:], r2_d[th][:].rearrange("c p -> (c p)")[None, :])
                nc.vector.tensor_scalar(m2row[:], m2row[:], 1e-5, None, op0=OP.max)
                rs2 = rowp.tile([1, 512], F32, tag="lscr")
                s2row = rowp.tile([1, 512], F32, tag="lrow")
                nc.vector.reciprocal_approx_accurate(s2row[:], m2row[:], rs2[:])
                nc.vector.tensor_scalar(s2row[:], s2row[:], 127.0, None, op0=OP.mult)
                i2row = rowp.tile([1, 512], F32, tag="lrow")
                nc.vector.reciprocal_approx_accurate(i2row[:], s2row[:], rs2[:])
                nc.vector.tensor_scalar(i2row[:], i2row[:], invw[:, 1:2], None, op0=OP.mult)
                return s2row, i2row

            def half_bcast(th, s2row, i2row):
                to = th * 512
                bcast_row(s2r, s2row, 512, off=to)
                bcast_row(i2r, i2row, 512, off=to)

            def q2_block(th, htile, kg):
                to = th * 512
                hsl = htile[:, kg * 512:(kg + 1) * 512]
                t2s = psc.tile([128, 512], F32, tag="hs")
                nc.vector.tensor_tensor(t2s[:], hsl, s2r[:, to:to + 512], op=OP.mult)
                nc.vector.tensor_scalar(hsl, t2s[:], MAGIC, MAGIC,
                                        op0=OP.add, op1=OP.subtract)

            # ============ MM1 half B, with rows(A) + Q2(A) hidden under it ===
            q2a = 0
            rowsA = [None]
            for idx, (ci, r, bi) in enumerate(blocks):
                mm1_block(ci, r, bi, 1, h1)
                if idx == 4:
                    rowsA[0] = half_rows(0)
                elif idx == 8:
                    half_bcast(0, *rowsA[0])
                elif idx >= 10 and q2a < HB:
                    for _ in range(3):
                        if q2a < HB:
                            q2_block(0, h0, q2a)
                            q2a += 1
            while q2a < HB:
                q2_block(0, h0, q2a)
                q2a += 1

            # ============ MM2 per half, d-outer ==============================
            def mm2_d(th, htile, d):
                to = th * 512
                ps = ps_mm.tile([128, 512], F32, tag="mm")
                for r in range(n_cores):
                    wv = pmm2.tile([128, HBL * 128], BF16, tag="wv")
                    nc.scalar.dma_start(wv[:], t2_g[r, d])
                    for k2 in range(HBL):
                        kg = r * HBL + k2
                        nc.tensor.matmul(ps[:], wv[:, k2 * 128:(k2 + 1) * 128],
                                         htile[:, kg * 512:(kg + 1) * 512],
                                         start=(kg == 0), stop=(kg == HB - 1))
                ot = pmm2.tile([128, 512], F32, tag="ot")
                nc.vector.tensor_tensor(ot[:], ps[:], i2r[:, to:to + 512], op=OP.mult)
                nc.gpsimd.dma_start(outT[d * 128:(d + 1) * 128, to:to + 512], ot[:])

            q2b = 0
            rowsB = [None]
            for d in range(DB):
                mm2_d(0, h0, d)
                if d == 1:
                    rowsB[0] = half_rows(1)
                elif d == 3:
                    half_bcast(1, *rowsB[0])
                elif d >= 4 and q2b < HB:
                    for _ in range(6):
                        if q2b < HB:
                            q2_block(1, h1, q2b)
                            q2b += 1
            while q2b < HB:
                q2_block(1, h1, q2b)
                q2b += 1

            for d in range(DB):
                mm2_d(1, h1, d)

    nc.compile()
    return nc


def _get_nc():
    if "nc" not in _cache:
        _cache["nc"] = _build()
    return _cache["nc"]


def _prep_inputs(x, w1, w2, gamma):
    x2d = np.ascontiguousarray(np.asarray(x, dtype=np.float32).reshape(NTOK, DIM))
    w1 = np.asarray(w1, dtype=np.float32)
    w2 = np.asarray(w2, dtype=np.float32)
    gamma = np.asarray(gamma, dtype=np.float32)
    w1T = np.ascontiguousarray(w1.T)          # [DIM, HID]
    w2T = np.ascontiguousarray(w2.T)          # [HID, DIM]
    gpt = np.ascontiguousarray(gamma.reshape(KT, 128).T)
    hs = HID // NCORES
    in_maps = []
    for c in range(NCORES):
        in_maps.append({
            "xT": np.ascontiguousarray(x2d[c * TOK:(c + 1) * TOK, :].T),
            "w1s": np.ascontiguousarray(w1T[:, c * hs:(c + 1) * hs]),
            "w2s": np.ascontiguousarray(w2T[c * hs:(c + 1) * hs, :]),
            "gpt": gpt,
        })
    return in_maps


def _run(in_maps, trace=False, **kw):
    nc = _get_nc()
    return bass_utils.run_bass_kernel_spmd(
        nc, in_maps, core_ids=list(range(NCORES)), trace=trace, **kw)


def kernel(x, w1, w2, gamma):
    in_maps = _prep_inputs(x, w1, w2, gamma)
    res = _run(in_maps, trace=False)
    out = np.empty((NTOK, DIM), dtype=np.float32)
    for c in range(NCORES):
        out[c * TOK:(c + 1) * TOK, :] = res.results[c]["outT"].T
    return out.reshape(B, S, DIM)
